# revision 1
# baseline (speedup 1.0000x reference)
import numpy as np
import jax
import jax.numpy as jnp
from jax.sharding import Mesh, PartitionSpec as P
from jax.experimental.shard_map import shard_map
from functools import partial

# Problem constants (hardcoded per spec nn_CAM_63548336112251)
H = W = 5
HW = 25
C = 640
PN = 25        # way*shot train samples
QN = 2000      # way*query test samples
N_CORES = 8
QL = QN // N_CORES  # 250 queries per core
BN_EPS = 1e-5
NORM_EPS = 1e-12

_cached = {}


def _l2n(x):
    n = jnp.sqrt(jnp.sum(x * x, axis=1, keepdims=True))
    return x / jnp.maximum(n, NORM_EPS)


def _shard_fn(ftrain, ftest_s, w1, b1, gamma, beta, mean, var, w2, b2):
    # ftrain: (25, C, 5, 5) replicated; ftest_s: (QL, C, 5, 5) local shard
    nq = QL
    Bn = PN * nq
    ftr = _l2n(ftrain)
    fte = _l2n(ftest_s)

    At = ftr.reshape(PN, C, HW).transpose(1, 0, 2).reshape(C, PN * HW)
    Bt = fte.reshape(nq, C, HW).transpose(1, 0, 2).reshape(C, nq * HW)
    cos = At.T @ Bt                                  # (625, nq*25)
    cos4 = cos.reshape(PN, HW, nq, HW)               # p, hw, q, xy
    cos_b = cos4.transpose(0, 2, 1, 3).reshape(Bn, HW, HW)  # b=(p,q), hw, xy

    def kernel_net(g):
        h1 = g @ w1.T + b1
        h1 = (h1 - mean) / jnp.sqrt(var + BN_EPS) * gamma + beta
        h1 = jax.nn.relu(h1)
        return jax.nn.relu(h1 @ w2.T + b2)

    # train attention branch: channels = train spatial (hw)
    g = cos_b.mean(axis=2)                           # (Bn, 25)
    k = kernel_net(g)
    att_b = jnp.einsum('bk,bkx->bx', k, cos_b)       # (Bn, 25)
    att_partial = att_b.reshape(PN, nq, HW).sum(axis=1)  # (25, 25) local sum over q

    # test attention branch: channels = test spatial (xy)
    cst = cos_b.transpose(0, 2, 1)                   # (Bn, xy, hw)
    gt = cos_b.mean(axis=1)                          # (Bn, 25) = mean over hw
    kt = kernel_net(gt)
    att_t = jnp.einsum('bk,bkh->bh', kt, cst)        # (Bn, hw)
    att_t = att_t.reshape(PN, nq, HW).transpose(1, 0, 2)  # (nq, 25 p, 25 hw)

    cls = att_t.reshape(nq, 5, 5, HW).mean(axis=3).mean(axis=2)  # (nq, way)
    cls_scores = jax.nn.softmax(cls, axis=1)

    am = att_t.mean(axis=1)                          # (nq, 25)
    am = jax.nn.softmax(am, axis=1).reshape(nq, 1, H, W)
    ftest_out = ftest_s * (am + 1.0)

    return att_partial, ftest_out, am, cls_scores


def _get_compiled():
    if 'fn' in _cached:
        return _cached['fn']
    devices = jax.devices()[:N_CORES]
    mesh = Mesh(np.asarray(devices), ('core',))
    rep = P()
    fn = jax.jit(shard_map(
        _shard_fn, mesh=mesh,
        in_specs=(rep, P('core'), rep, rep, rep, rep, rep, rep, rep, rep),
        out_specs=(P('core'), P('core'), P('core'), P('core')),
        check_rep=False,
    ))
    _cached['fn'] = fn
    return fn


def kernel(ftrain, ftest, data_shot, data_query, ytrain, ytest,
           conv1_w, conv1_b, bn_gamma, bn_beta, bn_mean, bn_var,
           conv2_w, conv2_b, way, shot, query, **_unused):
    ftrain = np.asarray(ftrain, np.float32)
    ftest = np.asarray(ftest, np.float32)
    w1 = np.asarray(conv1_w, np.float32).reshape(5, 25)
    w2 = np.asarray(conv2_w, np.float32).reshape(25, 5)
    args = (ftrain, ftest, w1, np.asarray(conv1_b, np.float32),
            np.asarray(bn_gamma, np.float32), np.asarray(bn_beta, np.float32),
            np.asarray(bn_mean, np.float32), np.asarray(bn_var, np.float32),
            w2, np.asarray(conv2_b, np.float32))

    fn = _get_compiled()
    att_parts, ftest_out, am, cls_scores = fn(*args)

    # finish the (tiny) train-attention branch on host: global mean over Q,
    # softmax over HW, then scale ftrain
    att_parts = np.asarray(att_parts).reshape(N_CORES, PN, HW)
    att = att_parts.sum(axis=0) / float(QN)           # (25, 25)
    att = att - att.max(axis=1, keepdims=True)
    e = np.exp(att)
    att = e / e.sum(axis=1, keepdims=True)
    att = att.reshape(PN, 1, H, W).astype(np.float32)
    ftrain_out = ftrain * (att + 1.0)

    return (np.asarray(ftrain_out, np.float32),
            np.asarray(ftest_out, np.float32),
            np.asarray(am, np.float32),
            np.asarray(cls_scores, np.float32))


# revision 2
# speedup vs baseline: 1.0120x; 1.0120x over previous
import numpy as np
import jax
import jax.numpy as jnp
from jax.sharding import Mesh, PartitionSpec as P
from jax.experimental.shard_map import shard_map
from functools import partial

# Problem constants (hardcoded per spec nn_CAM_63548336112251)
H = W = 5
HW = 25
C = 640
PN = 25        # way*shot train samples
QN = 2000      # way*query test samples
N_CORES = 8
QL = QN // N_CORES  # 250 queries per core
BN_EPS = 1e-5
NORM_EPS = 1e-12

_cached = {}


def _l2n(x):
    n = jnp.sqrt(jnp.sum(x * x, axis=1, keepdims=True))
    return x / jnp.maximum(n, NORM_EPS)


def _shard_fn(ftrain, ftest_s, w1, b1, gamma, beta, mean, var, w2, b2):
    # ftrain: (25, C, 5, 5) replicated; ftest_s: (QL, C, 5, 5) local shard
    nq = QL
    # stay in (n, C, HW) layout throughout — no big physical transposes
    ftr = ftrain.reshape(PN, C, HW)
    fte = ftest_s.reshape(nq, C, HW)
    ftr = ftr / jnp.maximum(jnp.sqrt((ftr * ftr).sum(1, keepdims=True)), NORM_EPS)
    fte = fte / jnp.maximum(jnp.sqrt((fte * fte).sum(1, keepdims=True)), NORM_EPS)

    # contract over C without pre-transposing: (p,c,h)·(q,c,x) -> (p,h,q,x)
    cos4 = jax.lax.dot_general(ftr, fte, (((1,), (1,)), ((), ())))

    def kernel_net(g):
        h1 = g @ w1.T + b1
        h1 = (h1 - mean) / jnp.sqrt(var + BN_EPS) * gamma + beta
        h1 = jax.nn.relu(h1)
        return jax.nn.relu(h1 @ w2.T + b2)

    # train attention branch: channels = train spatial (hw)
    g = cos4.mean(axis=3).transpose(0, 2, 1).reshape(PN * nq, HW)
    k = kernel_net(g).reshape(PN, nq, HW)
    att_b = jnp.einsum('pqh,phqx->pqx', k, cos4)     # (p, q, xy)
    att_partial = att_b.sum(axis=1)                  # (25, 25) local sum over q

    # test attention branch: channels = test spatial (xy)
    gt = cos4.mean(axis=1).reshape(PN * nq, HW)      # mean over hw -> (p,q,xy)
    kt = kernel_net(gt).reshape(PN, nq, HW)
    att_t = jnp.einsum('pqx,phqx->qph', kt, cos4)    # (nq, 25 p, 25 hw)

    cls = att_t.reshape(nq, 5, 5, HW).mean(axis=3).mean(axis=2)  # (nq, way)
    cls_scores = jax.nn.softmax(cls, axis=1)

    am = att_t.mean(axis=1)                          # (nq, 25)
    am = jax.nn.softmax(am, axis=1).reshape(nq, 1, H, W)
    ftest_out = ftest_s * (am + 1.0)

    return att_partial, ftest_out, am, cls_scores


def _get_compiled():
    if 'fn' in _cached:
        return _cached['fn']
    devices = jax.devices()[:N_CORES]
    mesh = Mesh(np.asarray(devices), ('core',))
    rep = P()
    fn = jax.jit(shard_map(
        _shard_fn, mesh=mesh,
        in_specs=(rep, P('core'), rep, rep, rep, rep, rep, rep, rep, rep),
        out_specs=(P('core'), P('core'), P('core'), P('core')),
        check_rep=False,
    ))
    _cached['fn'] = fn
    return fn


def kernel(ftrain, ftest, data_shot, data_query, ytrain, ytest,
           conv1_w, conv1_b, bn_gamma, bn_beta, bn_mean, bn_var,
           conv2_w, conv2_b, way, shot, query, **_unused):
    ftrain = np.asarray(ftrain, np.float32)
    ftest = np.asarray(ftest, np.float32)
    w1 = np.asarray(conv1_w, np.float32).reshape(5, 25)
    w2 = np.asarray(conv2_w, np.float32).reshape(25, 5)
    args = (ftrain, ftest, w1, np.asarray(conv1_b, np.float32),
            np.asarray(bn_gamma, np.float32), np.asarray(bn_beta, np.float32),
            np.asarray(bn_mean, np.float32), np.asarray(bn_var, np.float32),
            w2, np.asarray(conv2_b, np.float32))

    fn = _get_compiled()
    att_parts, ftest_out, am, cls_scores = fn(*args)

    # finish the (tiny) train-attention branch on host: global mean over Q,
    # softmax over HW, then scale ftrain
    att_parts = np.asarray(att_parts).reshape(N_CORES, PN, HW)
    att = att_parts.sum(axis=0) / float(QN)           # (25, 25)
    att = att - att.max(axis=1, keepdims=True)
    e = np.exp(att)
    att = e / e.sum(axis=1, keepdims=True)
    att = att.reshape(PN, 1, H, W).astype(np.float32)
    ftrain_out = ftrain * (att + 1.0)

    return (np.asarray(ftrain_out, np.float32),
            np.asarray(ftest_out, np.float32),
            np.asarray(am, np.float32),
            np.asarray(cls_scores, np.float32))
